# revision 15
# baseline (speedup 1.0000x reference)
"""CIN (Compressed Interaction Network) kernel for Trainium2, 8 NeuronCores.

Reference computation (per sample b, NFIELD=64, NEMB=64, NFILTER=128, 3 layers):
    xk_{l+1}[o, e] = relu( sum_{f,c} W_l[o, f*C+c] * x0[f, e] * xk_l[c, e] )
    pooled_l = sum_e xk_{l+1};  y = concat(pooled) @ Wa.T

Strategy:
  - Data-parallel over batch: 32 samples/core, free axis J = 32*64 = 2048 (b-major,
    e-minor). Columns are independent through all layers; only the final pooled
    sum groups by b.
  - Per layer the GEMM is out = W @ H with H[(f,c), j] = x0[f,j] * xk[c,j]
    (Khatri-Rao column structure). H is materialized K-tile by K-tile in bf16 by
    DVE tensor_tensor with plain 2D unit-stride APs (DVE 2x_1P perf mode).
  - Layer 0 is symmetric (xk = x0): W0 is host-folded onto upper-triangle
    (f<=c) pairs, K = 2080 -> 17 K-tiles (vs 32), and both TT operands are
    host-gathered arrays (x0pack_f/x0pack_c) loaded straight from DRAM.
  - Layers 1-2: the x0_f modulator rows are partition-replicated in "hex"
    tiles of 16 fields: one seed DMA (32 partitions, from a host-side
    32x-replicated x0rep32) + 2 partition-doubling SBUF->SBUF DMAs.
    DMA issue is wave-interleaved across hex tiles to avoid FIFO
    head-of-line blocking on the Sync queue.
  - PE runs bf16 matmuls (N=512) accumulating in PSUM; ScalarE applies ReLU
    4x into a repeated next-layer input xk4; VectorE reduces pooled in fp32.
  - Weights host-pre-transposed to (K, O) bf16; W1/W2 loaded via the GPSIMD
    DMA queue so they don't block the Sync queue at startup.
"""

import sys

if "/opt/trn_rl_repo" not in sys.path:
    sys.path.insert(0, "/opt/trn_rl_repo")

import numpy as np
import ml_dtypes

B, F, E, O = 256, 64, 64, 128
NCORES = 8
BC = B // NCORES          # samples per core
J = BC * E                # free columns per core
JB = 512                  # free-block size (one PSUM bank)
NJ = J // JB              # 4 free blocks
KT0 = 17                  # layer-0 K-tiles (packed symmetric, 2176 = 17*128)
K0 = KT0 * 128
KT = [KT0, 64, 64]

_BF16 = ml_dtypes.bfloat16
_STATE = {}

# layer-0 packed pair enumeration (f <= c), padded to K0 with (0, 0)
_PAIRS = [(f, c) for f in range(F) for c in range(f, F)]
_F_IDX = np.array([p[0] for p in _PAIRS] + [0] * (K0 - len(_PAIRS)), np.int64)
_C_IDX = np.array([p[1] for p in _PAIRS] + [0] * (K0 - len(_PAIRS)), np.int64)


def _build_nc():
    import concourse.bass as bass
    import concourse.tile as tile
    import concourse.mybir as mybir
    from concourse import bacc

    dt = mybir.dt
    nc = bacc.Bacc("TRN2", target_bir_lowering=False, debug=False)

    # host-relayout arrays: contiguous per DMA destination partition
    x0seed = nc.dram_tensor(
        "x0seed", [NJ * 4, 32, 16 * JB], dt.bfloat16, kind="ExternalInput"
    )
    x0packf = nc.dram_tensor(
        "x0packf", [NJ, 128, KT0 * JB], dt.bfloat16, kind="ExternalInput"
    )
    x0packc = nc.dram_tensor(
        "x0packc", [NJ, 128, KT0 * JB], dt.bfloat16, kind="ExternalInput"
    )
    w0t = nc.dram_tensor("w0t", [K0, O], dt.bfloat16, kind="ExternalInput")
    w1t = nc.dram_tensor("w1t", [F * O, O], dt.bfloat16, kind="ExternalInput")
    w2t = nc.dram_tensor("w2t", [F * O, O], dt.bfloat16, kind="ExternalInput")
    wa = nc.dram_tensor("wa", [O, 3], dt.float32, kind="ExternalInput")
    y = nc.dram_tensor("y", [1, BC], dt.float32, kind="ExternalOutput")

    HEXW = 16 * JB            # free width of a 16-field modulator tile
    PKW = KT0 * JB            # free width of a packed layer-0 operand tile

    with tile.TileContext(nc) as tc:
        with (
            tc.tile_pool(name="wpool", bufs=1) as wpool,
            tc.tile_pool(name="xpool", bufs=1) as xpool,
            tc.tile_pool(name="modpool", bufs=4) as modpool,
            tc.tile_pool(name="packpool", bufs=1) as packpool,
            tc.tile_pool(name="hpool", bufs=4) as hpool,
            tc.tile_pool(name="xkpool", bufs=2) as xkpool,
            tc.tile_pool(name="psum", bufs=2, space="PSUM") as psum_pool,
            tc.tile_pool(name="psumy", bufs=1, space="PSUM") as psumy_pool,
        ):
            # --- static loads (GPSIMD queue; Sync queue is for modulators) ----
            wa_sb = xpool.tile([O, 3], dt.float32, tag="wa")
            nc.gpsimd.dma_start(wa_sb[:], wa[:])
            w_sb = []
            for li, (wd, kt) in enumerate(zip((w0t, w1t, w2t), KT)):
                w = wpool.tile([128, kt, O], dt.bfloat16, tag=f"w{li}", name=f"w{li}")
                nc.gpsimd.dma_start(w[:], wd[:].rearrange("(t p) o -> p t o", p=128))
                w_sb.append(w)
            pooled = [
                xpool.tile([O, BC], dt.float32, tag=f"pooled{l}", name=f"pooled{l}")
                for l in range(3)
            ]

            # --- main loop over free blocks ----------------------------------
            for jj in range(NJ):
                jsl = slice(JB * jj, JB * (jj + 1))
                # layer-0 packed operand tiles (contiguous per-partition loads)
                p0f = packpool.tile([128, PKW], dt.bfloat16, tag="p0f", name=f"p0f{jj}")
                p0c = packpool.tile([128, PKW], dt.bfloat16, tag="p0c", name=f"p0c{jj}")
                nc.gpsimd.dma_start(p0f[:], x0packf[jj])
                nc.gpsimd.dma_start(p0c[:], x0packc[jj])
                # modulator hex tiles for layers 1-2:
                #   mh[p, 512*i + e] = x0[16*hx + i, jsl][e] for every p
                # wave-interleaved issue: seeds for all hexes, then doubling waves
                mhs = []
                for hx in range(4):
                    mh = modpool.tile(
                        [128, HEXW], dt.bfloat16, tag="mod", name=f"mh{jj}_{hx}"
                    )
                    nc.sync.dma_start(mh[0:32, :], x0seed[4 * jj + hx])
                    mhs.append(mh)
                for hx in range(4):
                    nc.sync.dma_start(mhs[hx][32:64, :], mhs[hx][0:32, :])
                for hx in range(4):
                    nc.sync.dma_start(mhs[hx][64:128, :], mhs[hx][0:64, :])

                xk4 = None
                for l in range(3):
                    kt = KT[l]
                    acc = psum_pool.tile(
                        [128, JB], dt.float32, tag="acc", name=f"acc{jj}_{l}"
                    )
                    if l == 0:
                        # 8 pair ops + 1 single op over 17 packed K-tiles
                        for s in range(9):
                            nk = 2 if s < 8 else 1
                            h = hpool.tile(
                                [128, 4 * JB], dt.bfloat16, tag="h", name=f"h0_{jj}_{s}"
                            )
                            w_ = JB * nk
                            nc.vector.tensor_tensor(
                                h[:, 0:w_],
                                p0c[:, 2 * JB * s : 2 * JB * s + w_],
                                p0f[:, 2 * JB * s : 2 * JB * s + w_],
                                op=mybir.AluOpType.mult,
                            )
                            for i in range(nk):
                                t = 2 * s + i
                                nc.tensor.matmul(
                                    acc[:], w_sb[0][:, t, :],
                                    h[:, JB * i : JB * (i + 1)],
                                    start=(t == 0), stop=(t == kt - 1),
                                )
                    else:
                        for hx in range(4):
                            for s in range(4):
                                h = hpool.tile(
                                    [128, 4 * JB], dt.bfloat16, tag="h",
                                    name=f"h{jj}_{l}_{hx}_{s}",
                                )
                                nc.vector.tensor_tensor(
                                    h[:], xk4[:],
                                    mhs[hx][:, 4 * JB * s : 4 * JB * (s + 1)],
                                    op=mybir.AluOpType.mult,
                                )
                                for i in range(4):
                                    t = 16 * hx + 4 * s + i
                                    nc.tensor.matmul(
                                        acc[:], w_sb[l][:, t, :],
                                        h[:, JB * i : JB * (i + 1)],
                                        start=(t == 0), stop=(t == kt - 1),
                                    )
                    # epilogue: relu 4x into xk4 (repeated next-layer input)
                    xk4_new = xkpool.tile(
                        [128, 4 * JB], dt.bfloat16, tag="xk4", name=f"xk4_{jj}_{l}"
                    )
                    for i in range(4):
                        nc.scalar.activation(
                            xk4_new[:, JB * i : JB * (i + 1)], acc[:],
                            mybir.ActivationFunctionType.Relu,
                        )
                    nc.vector.tensor_reduce(
                        pooled[l][:, 8 * jj : 8 * jj + 8],
                        xk4_new[:, 0:JB].rearrange("p (b e) -> p b e", e=E),
                        axis=mybir.AxisListType.X,
                        op=mybir.AluOpType.add,
                    )
                    xk4 = xk4_new

            # --- head: y[b] = sum_l wa[:, l] . pooled[l][:, b] ----------------
            yac = psumy_pool.tile([1, BC], dt.float32, tag="yac")
            for l in range(3):
                nc.tensor.matmul(
                    yac[:], wa_sb[:, l : l + 1], pooled[l][:],
                    start=(l == 0), stop=(l == 2),
                )
            y_sb = xpool.tile([1, BC], dt.float32, tag="ysb")
            nc.scalar.copy(y_sb[:], yac[:])
            nc.sync.dma_start(y[:], y_sb[:])

    nc.finalize()
    return nc


def _get_nc():
    if "nc" not in _STATE:
        _STATE["nc"] = _build_nc()
    return _STATE["nc"]


def _pack_w0(W0):
    # fold symmetric (f, c) weight pairs onto f <= c; pad to K0 with zeros
    w = np.asarray(W0, np.float32).reshape(O, F, F)
    wp = np.zeros((O, K0), np.float32)
    k = 0
    for f in range(F):
        wp[:, k] = w[:, f, f]
        k += 1
        n = F - f - 1
        if n:
            wp[:, k : k + n] = w[:, f, f + 1 :] + w[:, f + 1 :, f]
            k += n
    return wp


def _prep_in_maps(x, W0, W1, W2, Wa):
    x = np.asarray(x, dtype=np.float32)
    w0t = np.ascontiguousarray(_pack_w0(W0).T).astype(_BF16)
    w1t = np.ascontiguousarray(np.asarray(W1, np.float32).T).astype(_BF16)
    w2t = np.ascontiguousarray(np.asarray(W2, np.float32).T).astype(_BF16)
    wa = np.ascontiguousarray(np.asarray(Wa, np.float32).reshape(3, O).T)
    def pack_gather(x0b, idx):
        g = x0b[idx]                                        # (K0, J)
        g = g.reshape(KT0, 128, NJ, JB).transpose(2, 1, 0, 3)
        return np.ascontiguousarray(g.reshape(NJ, 128, KT0 * JB))

    in_maps = []
    for c in range(NCORES):
        xc = x[c * BC : (c + 1) * BC]                       # (BC, F, E)
        x0 = np.ascontiguousarray(xc.transpose(1, 0, 2).reshape(F, J))
        x0b = x0.astype(_BF16)
        # seed blocks: x0seed[4*jj+hx] = x0[16hx:16hx+16, jj-block] flattened,
        # replicated across 32 partitions
        x0r = x0b.reshape(F, NJ, JB)
        seeds = np.empty((NJ * 4, 32, 16 * JB), _BF16)
        for jj in range(NJ):
            for hx in range(4):
                blk = x0r[16 * hx : 16 * hx + 16, jj].reshape(1, 16 * JB)
                seeds[4 * jj + hx] = np.broadcast_to(blk, (32, 16 * JB))
        in_maps.append(
            {
                "x0seed": seeds,
                "x0packf": pack_gather(x0b, _F_IDX),
                "x0packc": pack_gather(x0b, _C_IDX),
                "w0t": w0t,
                "w1t": w1t,
                "w2t": w2t,
                "wa": wa,
            }
        )
    return in_maps


def _run(inputs, trace=False, **kwargs):
    from concourse.bass_utils import run_bass_kernel_spmd

    nc = _get_nc()
    in_maps = _prep_in_maps(**inputs)
    res = run_bass_kernel_spmd(
        nc, in_maps, core_ids=list(range(NCORES)), trace=trace, **kwargs
    )
    y = np.concatenate(
        [np.asarray(r["y"], np.float32).reshape(BC) for r in res.results]
    )
    return y, res


def kernel(**inputs) -> np.ndarray:
    y, _ = _run(inputs, trace=False)
    return y


# revision 19
# speedup vs baseline: 1.0257x; 1.0257x over previous
"""CIN (Compressed Interaction Network) kernel for Trainium2, 8 NeuronCores.

Reference computation (per sample b, NFIELD=64, NEMB=64, NFILTER=128, 3 layers):
    xk_{l+1}[o, e] = relu( sum_{f,c} W_l[o, f*C+c] * x0[f, e] * xk_l[c, e] )
    pooled_l = sum_e xk_{l+1};  y = concat(pooled) @ Wa.T

Strategy:
  - Data-parallel over batch: 32 samples/core, free axis J = 32*64 = 2048 (b-major,
    e-minor). Columns are independent through all layers; only the final pooled
    sum groups by b.
  - Per layer the GEMM is out = W @ H with H[(f,c), j] = x0[f,j] * xk[c,j]
    (Khatri-Rao column structure). H is materialized K-tile by K-tile in bf16 by
    DVE tensor_tensor with plain 2D unit-stride APs (DVE 2x_1P perf mode).
  - Layer 0 is symmetric (xk = x0): W0 is host-folded onto upper-triangle
    (f<=c) pairs, K = 2080 -> 17 K-tiles (vs 32), and both TT operands are
    host-gathered arrays (x0pack_f/x0pack_c) loaded straight from DRAM.
  - Layers 1-2: the x0_f modulator rows are partition-replicated in "hex"
    tiles of 16 fields: one seed DMA (32 partitions, from a host-side
    32x-replicated x0rep32) + 2 partition-doubling SBUF->SBUF DMAs.
    DMA issue is wave-interleaved across hex tiles to avoid FIFO
    head-of-line blocking on the Sync queue.
  - PE runs bf16 matmuls (N=512) accumulating in PSUM; ScalarE applies ReLU
    4x into a repeated next-layer input xk4; VectorE reduces pooled in fp32.
  - Weights host-pre-transposed to (K, O) bf16; W1/W2 loaded via the GPSIMD
    DMA queue so they don't block the Sync queue at startup.
"""

import sys

if "/opt/trn_rl_repo" not in sys.path:
    sys.path.insert(0, "/opt/trn_rl_repo")

import numpy as np
import ml_dtypes

B, F, E, O = 256, 64, 64, 128
NCORES = 8
BC = B // NCORES          # samples per core
J = BC * E                # free columns per core
JB = 512                  # free-block size (one PSUM bank)
NJ = J // JB              # 4 free blocks
KT0 = 17                  # layer-0 K-tiles (packed symmetric, 2176 = 17*128)
K0 = KT0 * 128
KT = [KT0, 64, 64]

_BF16 = ml_dtypes.bfloat16
_STATE = {}

# layer-0 packed pair enumeration (f <= c), padded to K0 with (0, 0)
_PAIRS = [(f, c) for f in range(F) for c in range(f, F)]
_F_IDX = np.array([p[0] for p in _PAIRS] + [0] * (K0 - len(_PAIRS)), np.int64)
_C_IDX = np.array([p[1] for p in _PAIRS] + [0] * (K0 - len(_PAIRS)), np.int64)


def _build_nc():
    import concourse.bass as bass
    import concourse.tile as tile
    import concourse.mybir as mybir
    from concourse import bacc

    dt = mybir.dt
    nc = bacc.Bacc("TRN2", target_bir_lowering=False, debug=False)

    # host-relayout arrays: contiguous per DMA destination partition
    x0seed = nc.dram_tensor(
        "x0seed", [NJ * 4, 32, 16 * JB], dt.bfloat16, kind="ExternalInput"
    )
    x0packf = nc.dram_tensor(
        "x0packf", [NJ, 128, KT0 * JB], dt.bfloat16, kind="ExternalInput"
    )
    x0packc = nc.dram_tensor(
        "x0packc", [NJ, 128, KT0 * JB], dt.bfloat16, kind="ExternalInput"
    )
    # weights pre-laid-out as [partition, ktile*O] (contiguous per partition)
    w0t = nc.dram_tensor("w0t", [128, KT0 * O], dt.bfloat16, kind="ExternalInput")
    w1t = nc.dram_tensor("w1t", [128, 64 * O], dt.bfloat16, kind="ExternalInput")
    w2t = nc.dram_tensor("w2t", [128, 64 * O], dt.bfloat16, kind="ExternalInput")
    wa = nc.dram_tensor("wa", [O, 3], dt.float32, kind="ExternalInput")
    y = nc.dram_tensor("y", [1, BC], dt.float32, kind="ExternalOutput")

    HEXW = 16 * JB            # free width of a 16-field modulator tile
    PKW = KT0 * JB            # free width of a packed layer-0 operand tile

    with tile.TileContext(nc) as tc:
        with (
            tc.tile_pool(name="wpool", bufs=1) as wpool,
            tc.tile_pool(name="xpool", bufs=1) as xpool,
            tc.tile_pool(name="modpool", bufs=4) as modpool,
            tc.tile_pool(name="packpool", bufs=1) as packpool,
            tc.tile_pool(name="hpool", bufs=4) as hpool,
            tc.tile_pool(name="xkpool", bufs=2) as xkpool,
            tc.tile_pool(name="psum", bufs=2, space="PSUM") as psum_pool,
            tc.tile_pool(name="psumy", bufs=1, space="PSUM") as psumy_pool,
        ):
            # --- static loads -------------------------------------------------
            wa_sb = xpool.tile([O, 3], dt.float32, tag="wa")
            nc.sync.dma_start(wa_sb[:], wa[:])
            w_sb = []
            for li, (wd, kt) in enumerate(zip((w0t, w1t, w2t), KT)):
                w = wpool.tile([128, kt, O], dt.bfloat16, tag=f"w{li}", name=f"w{li}")
                nc.sync.dma_start(w[:].rearrange("p t o -> p (t o)"), wd[:])
                w_sb.append(w)
            pooled = [
                xpool.tile([O, BC], dt.float32, tag=f"pooled{l}", name=f"pooled{l}")
                for l in range(3)
            ]

            # --- main loop over free blocks ----------------------------------
            for jj in range(NJ):
                jsl = slice(JB * jj, JB * (jj + 1))
                # layer-0 packed operand tiles (contiguous per-partition loads)
                p0f = packpool.tile([128, PKW], dt.bfloat16, tag="p0f", name=f"p0f{jj}")
                p0c = packpool.tile([128, PKW], dt.bfloat16, tag="p0c", name=f"p0c{jj}")
                nc.sync.dma_start(p0f[:], x0packf[jj])
                nc.sync.dma_start(p0c[:], x0packc[jj])
                # modulator hex tiles for layers 1-2:
                #   mh[p, 512*i + e] = x0[16*hx + i, jsl][e] for every p
                # wave-interleaved issue: seeds for all hexes, then doubling waves
                mhs = []
                for hx in range(4):
                    mh = modpool.tile(
                        [128, HEXW], dt.bfloat16, tag="mod", name=f"mh{jj}_{hx}"
                    )
                    nc.sync.dma_start(mh[0:32, :], x0seed[4 * jj + hx])
                    mhs.append(mh)
                for hx in range(4):
                    nc.sync.dma_start(mhs[hx][32:64, :], mhs[hx][0:32, :])
                for hx in range(4):
                    nc.sync.dma_start(mhs[hx][64:128, :], mhs[hx][0:64, :])

                xk4 = None
                for l in range(3):
                    kt = KT[l]
                    acc = psum_pool.tile(
                        [128, JB], dt.float32, tag="acc", name=f"acc{jj}_{l}"
                    )
                    if l == 0:
                        # 8 pair ops + 1 single op over 17 packed K-tiles
                        for s in range(9):
                            nk = 2 if s < 8 else 1
                            h = hpool.tile(
                                [128, 4 * JB], dt.bfloat16, tag="h", name=f"h0_{jj}_{s}"
                            )
                            w_ = JB * nk
                            nc.vector.tensor_tensor(
                                h[:, 0:w_],
                                p0c[:, 2 * JB * s : 2 * JB * s + w_],
                                p0f[:, 2 * JB * s : 2 * JB * s + w_],
                                op=mybir.AluOpType.mult,
                            )
                            for i in range(nk):
                                t = 2 * s + i
                                nc.tensor.matmul(
                                    acc[:], w_sb[0][:, t, :],
                                    h[:, JB * i : JB * (i + 1)],
                                    start=(t == 0), stop=(t == kt - 1),
                                )
                    else:
                        for hx in range(4):
                            for s in range(4):
                                h = hpool.tile(
                                    [128, 4 * JB], dt.bfloat16, tag="h",
                                    name=f"h{jj}_{l}_{hx}_{s}",
                                )
                                nc.vector.tensor_tensor(
                                    h[:], xk4[:],
                                    mhs[hx][:, 4 * JB * s : 4 * JB * (s + 1)],
                                    op=mybir.AluOpType.mult,
                                )
                                for i in range(4):
                                    t = 16 * hx + 4 * s + i
                                    nc.tensor.matmul(
                                        acc[:], w_sb[l][:, t, :],
                                        h[:, JB * i : JB * (i + 1)],
                                        start=(t == 0), stop=(t == kt - 1),
                                    )
                    # epilogue: relu 4x into xk4 (repeated next-layer input)
                    xk4_new = xkpool.tile(
                        [128, 4 * JB], dt.bfloat16, tag="xk4", name=f"xk4_{jj}_{l}"
                    )
                    for i in range(4):
                        nc.scalar.activation(
                            xk4_new[:, JB * i : JB * (i + 1)], acc[:],
                            mybir.ActivationFunctionType.Relu,
                        )
                    nc.vector.tensor_reduce(
                        pooled[l][:, 8 * jj : 8 * jj + 8],
                        xk4_new[:, 0:JB].rearrange("p (b e) -> p b e", e=E),
                        axis=mybir.AxisListType.X,
                        op=mybir.AluOpType.add,
                    )
                    xk4 = xk4_new

            # --- head: y[b] = sum_l wa[:, l] . pooled[l][:, b] ----------------
            yac = psumy_pool.tile([1, BC], dt.float32, tag="yac")
            for l in range(3):
                nc.tensor.matmul(
                    yac[:], wa_sb[:, l : l + 1], pooled[l][:],
                    start=(l == 0), stop=(l == 2),
                )
            y_sb = xpool.tile([1, BC], dt.float32, tag="ysb")
            nc.scalar.copy(y_sb[:], yac[:])
            nc.sync.dma_start(y[:], y_sb[:])

    nc.finalize()
    return nc


def _get_nc():
    if "nc" not in _STATE:
        _STATE["nc"] = _build_nc()
    return _STATE["nc"]


def _pack_w0(W0):
    # fold symmetric (f, c) weight pairs onto f <= c; pad to K0 with zeros
    w = np.asarray(W0, np.float32).reshape(O, F, F)
    wp = np.zeros((O, K0), np.float32)
    k = 0
    for f in range(F):
        wp[:, k] = w[:, f, f]
        k += 1
        n = F - f - 1
        if n:
            wp[:, k : k + n] = w[:, f, f + 1 :] + w[:, f + 1 :, f]
            k += n
    return wp


def _prep_in_maps(x, W0, W1, W2, Wa):
    x = np.asarray(x, dtype=np.float32)

    def w_layout(wt):
        # (K, O) -> (128, KT*O): row p holds [Wt[128t+p, :] for t in 0..KT)
        K = wt.shape[0]
        return np.ascontiguousarray(
            wt.reshape(K // 128, 128, O).transpose(1, 0, 2).reshape(128, -1)
        )

    w0t = w_layout(_pack_w0(W0).T).astype(_BF16)
    w1t = w_layout(np.ascontiguousarray(np.asarray(W1, np.float32).T)).astype(_BF16)
    w2t = w_layout(np.ascontiguousarray(np.asarray(W2, np.float32).T)).astype(_BF16)
    wa = np.ascontiguousarray(np.asarray(Wa, np.float32).reshape(3, O).T)
    def pack_gather(x0b, idx):
        g = x0b[idx]                                        # (K0, J)
        g = g.reshape(KT0, 128, NJ, JB).transpose(2, 1, 0, 3)
        return np.ascontiguousarray(g.reshape(NJ, 128, KT0 * JB))

    in_maps = []
    for c in range(NCORES):
        xc = x[c * BC : (c + 1) * BC]                       # (BC, F, E)
        x0 = np.ascontiguousarray(xc.transpose(1, 0, 2).reshape(F, J))
        x0b = x0.astype(_BF16)
        # seed blocks: x0seed[4*jj+hx] = x0[16hx:16hx+16, jj-block] flattened,
        # replicated across 32 partitions
        x0r = x0b.reshape(F, NJ, JB)
        seeds = np.empty((NJ * 4, 32, 16 * JB), _BF16)
        for jj in range(NJ):
            for hx in range(4):
                blk = x0r[16 * hx : 16 * hx + 16, jj].reshape(1, 16 * JB)
                seeds[4 * jj + hx] = np.broadcast_to(blk, (32, 16 * JB))
        in_maps.append(
            {
                "x0seed": seeds,
                "x0packf": pack_gather(x0b, _F_IDX),
                "x0packc": pack_gather(x0b, _C_IDX),
                "w0t": w0t,
                "w1t": w1t,
                "w2t": w2t,
                "wa": wa,
            }
        )
    return in_maps


def _run(inputs, trace=False, **kwargs):
    from concourse.bass_utils import run_bass_kernel_spmd

    nc = _get_nc()
    in_maps = _prep_in_maps(**inputs)
    res = run_bass_kernel_spmd(
        nc, in_maps, core_ids=list(range(NCORES)), trace=trace, **kwargs
    )
    y = np.concatenate(
        [np.asarray(r["y"], np.float32).reshape(BC) for r in res.results]
    )
    return y, res


def kernel(**inputs) -> np.ndarray:
    y, _ = _run(inputs, trace=False)
    return y


# revision 25
# speedup vs baseline: 1.0626x; 1.0360x over previous
"""CIN (Compressed Interaction Network) kernel for Trainium2, 8 NeuronCores.

Reference computation (per sample b, NFIELD=64, NEMB=64, NFILTER=128, 3 layers):
    xk_{l+1}[o, e] = relu( sum_{f,c} W_l[o, f*C+c] * x0[f, e] * xk_l[c, e] )
    pooled_l = sum_e xk_{l+1};  y = concat(pooled) @ Wa.T

Strategy:
  - Data-parallel over batch: 32 samples/core, free axis J = 32*64 = 2048 (b-major,
    e-minor). Columns are independent through all layers; only the final pooled
    sum groups by b.
  - Per layer the GEMM is out = W @ H with H[(f,c), j] = x0[f,j] * xk[c,j]
    (Khatri-Rao column structure). H is materialized K-tile by K-tile in bf16 by
    DVE tensor_tensor with plain 2D unit-stride APs (DVE 2x_1P perf mode).
  - Layer 0 is symmetric (xk = x0): W0 is host-folded onto upper-triangle
    (f<=c) pairs, K = 2080 -> 17 K-tiles (vs 32), and both TT operands are
    host-gathered arrays (x0pack_f/x0pack_c) loaded straight from DRAM.
  - Layers 1-2: the x0_f modulator rows are partition-replicated in "hex"
    tiles of 16 fields: one seed DMA (32 partitions, from a host-side
    32x-replicated x0rep32) + 2 partition-doubling SBUF->SBUF DMAs.
    DMA issue is wave-interleaved across hex tiles to avoid FIFO
    head-of-line blocking on the Sync queue.
  - PE runs bf16 matmuls (N=512) accumulating in PSUM; ScalarE applies ReLU
    4x into a repeated next-layer input xk4; VectorE reduces pooled in fp32.
  - Weights host-pre-transposed to (K, O) bf16; W1/W2 loaded via the GPSIMD
    DMA queue so they don't block the Sync queue at startup.
"""

import sys

if "/opt/trn_rl_repo" not in sys.path:
    sys.path.insert(0, "/opt/trn_rl_repo")

import numpy as np
import ml_dtypes

B, F, E, O = 256, 64, 64, 128
NCORES = 8
BC = B // NCORES          # samples per core
J = BC * E                # free columns per core
JB = 512                  # free-block size (one PSUM bank)
NJ = J // JB              # 4 free blocks
KT0 = 17                  # layer-0 K-tiles (packed symmetric, 2176 = 17*128)
K0 = KT0 * 128
KT = [KT0, 64, 64]

_BF16 = ml_dtypes.bfloat16
_STATE = {}

# layer-0 packed pair enumeration (f <= c), padded to K0 with (0, 0)
_PAIRS = [(f, c) for f in range(F) for c in range(f, F)]
_F_IDX = np.array([p[0] for p in _PAIRS] + [0] * (K0 - len(_PAIRS)), np.int64)
_C_IDX = np.array([p[1] for p in _PAIRS] + [0] * (K0 - len(_PAIRS)), np.int64)


def _build_nc():
    import concourse.bass as bass
    import concourse.tile as tile
    import concourse.mybir as mybir
    from concourse import bacc

    dt = mybir.dt
    nc = bacc.Bacc("TRN2", target_bir_lowering=False, debug=False)

    # host-relayout arrays: contiguous per DMA destination partition
    x0seed = nc.dram_tensor(
        "x0seed", [NJ * 4, 32, 16 * JB], dt.bfloat16, kind="ExternalInput"
    )
    x0packf = nc.dram_tensor(
        "x0packf", [NJ, 128, KT0 * JB], dt.bfloat16, kind="ExternalInput"
    )
    x0packc = nc.dram_tensor(
        "x0packc", [NJ, 128, KT0 * JB], dt.bfloat16, kind="ExternalInput"
    )
    # weights pre-laid-out as [partition, ktile*O] (contiguous per partition)
    w0t = nc.dram_tensor("w0t", [128, KT0 * O], dt.bfloat16, kind="ExternalInput")
    w1t = nc.dram_tensor("w1t", [128, 64 * O], dt.bfloat16, kind="ExternalInput")
    w2t = nc.dram_tensor("w2t", [128, 64 * O], dt.bfloat16, kind="ExternalInput")
    wa = nc.dram_tensor("wa", [O, 3], dt.float32, kind="ExternalInput")
    y = nc.dram_tensor("y", [1, BC], dt.float32, kind="ExternalOutput")

    HEXW = 16 * JB            # free width of a 16-field modulator tile
    PKW = KT0 * JB            # free width of a packed layer-0 operand tile

    with tile.TileContext(nc) as tc:
        with (
            tc.tile_pool(name="wpool", bufs=1) as wpool,
            tc.tile_pool(name="xpool", bufs=1) as xpool,
            tc.tile_pool(name="modpool", bufs=5) as modpool,
            tc.tile_pool(name="packpool", bufs=1) as packpool,
            tc.tile_pool(name="hpool", bufs=6) as hpool,
            tc.tile_pool(name="xkpool", bufs=2) as xkpool,
            tc.tile_pool(name="psum", bufs=2, space="PSUM") as psum_pool,
            tc.tile_pool(name="psumy", bufs=1, space="PSUM") as psumy_pool,
        ):
            # --- static loads -------------------------------------------------
            wa_sb = xpool.tile([O, 3], dt.float32, tag="wa")
            nc.sync.dma_start(wa_sb[:], wa[:])
            w_sb = []
            for li, (wd, kt) in enumerate(zip((w0t, w1t, w2t), KT)):
                w = wpool.tile([128, kt, O], dt.bfloat16, tag=f"w{li}", name=f"w{li}")
                nc.sync.dma_start(w[:].rearrange("p t o -> p (t o)"), wd[:])
                w_sb.append(w)
            pooled = [
                xpool.tile([O, BC], dt.float32, tag=f"pooled{l}", name=f"pooled{l}")
                for l in range(3)
            ]

            # --- main loop over free blocks ----------------------------------
            tt_rr = [0]
            for jj in range(NJ):
                jsl = slice(JB * jj, JB * (jj + 1))
                # layer-0 packed operand tiles (contiguous per-partition loads)
                p0f = packpool.tile([128, PKW], dt.bfloat16, tag="p0f", name=f"p0f{jj}")
                p0c = packpool.tile([128, PKW], dt.bfloat16, tag="p0c", name=f"p0c{jj}")
                nc.sync.dma_start(p0f[:], x0packf[jj])
                nc.sync.dma_start(p0c[:], x0packc[jj])
                # modulator hex tiles for layers 1-2:
                #   mh[p, 512*i + e] = x0[16*hx + i, jsl][e] for every p
                # chain-major issue: each hex's seed + doublings together, so a
                # chain blocked on its slot doesn't head-of-line-block the next
                # chain's transfers on the Sync queue.
                mhs = []
                for hx in range(4):
                    mh = modpool.tile(
                        [128, HEXW], dt.bfloat16, tag="mod", name=f"mh{jj}_{hx}"
                    )
                    nc.sync.dma_start(mh[0:32, :], x0seed[4 * jj + hx])
                    nc.sync.dma_start(mh[32:64, :], mh[0:32, :])
                    nc.sync.dma_start(mh[64:128, :], mh[0:64, :])
                    mhs.append(mh)

                xk4 = None
                for l in range(3):
                    kt = KT[l]
                    acc = psum_pool.tile(
                        [128, JB], dt.float32, tag="acc", name=f"acc{jj}_{l}"
                    )
                    if l == 0:
                        # 8 pair ops + 1 single op over 17 packed K-tiles
                        for s in range(9):
                            nk = 2 if s < 8 else 1
                            h = hpool.tile(
                                [128, 4 * JB], dt.bfloat16, tag="h", name=f"h0_{jj}_{s}"
                            )
                            w_ = JB * nk
                            tt_rr[0] += 1
                            eng = nc.gpsimd if tt_rr[0] % 5 == 0 else nc.vector
                            eng.tensor_tensor(
                                h[:, 0:w_],
                                p0c[:, 2 * JB * s : 2 * JB * s + w_],
                                p0f[:, 2 * JB * s : 2 * JB * s + w_],
                                op=mybir.AluOpType.mult,
                            )
                            for i in range(nk):
                                t = 2 * s + i
                                nc.tensor.matmul(
                                    acc[:], w_sb[0][:, t, :],
                                    h[:, JB * i : JB * (i + 1)],
                                    start=(t == 0), stop=(t == kt - 1),
                                )
                    else:
                        for hx in range(4):
                            for s in range(4):
                                h = hpool.tile(
                                    [128, 4 * JB], dt.bfloat16, tag="h",
                                    name=f"h{jj}_{l}_{hx}_{s}",
                                )
                                tt_rr[0] += 1
                                eng = nc.gpsimd if tt_rr[0] % 5 == 0 else nc.vector
                                eng.tensor_tensor(
                                    h[:], xk4[:],
                                    mhs[hx][:, 4 * JB * s : 4 * JB * (s + 1)],
                                    op=mybir.AluOpType.mult,
                                )
                                for i in range(4):
                                    t = 16 * hx + 4 * s + i
                                    nc.tensor.matmul(
                                        acc[:], w_sb[l][:, t, :],
                                        h[:, JB * i : JB * (i + 1)],
                                        start=(t == 0), stop=(t == kt - 1),
                                    )
                    # epilogue: relu 4x into xk4 (repeated next-layer input);
                    # the last layer only feeds pooled, one slice suffices
                    xk4_new = xkpool.tile(
                        [128, 4 * JB], dt.bfloat16, tag="xk4", name=f"xk4_{jj}_{l}"
                    )
                    for i in range(4 if l < 2 else 1):
                        nc.scalar.activation(
                            xk4_new[:, JB * i : JB * (i + 1)], acc[:],
                            mybir.ActivationFunctionType.Relu,
                        )
                    nc.vector.tensor_reduce(
                        pooled[l][:, 8 * jj : 8 * jj + 8],
                        xk4_new[:, 0:JB].rearrange("p (b e) -> p b e", e=E),
                        axis=mybir.AxisListType.X,
                        op=mybir.AluOpType.add,
                    )
                    xk4 = xk4_new

            # --- head: y[b] = sum_l wa[:, l] . pooled[l][:, b] ----------------
            yac = psumy_pool.tile([1, BC], dt.float32, tag="yac")
            for l in range(3):
                nc.tensor.matmul(
                    yac[:], wa_sb[:, l : l + 1], pooled[l][:],
                    start=(l == 0), stop=(l == 2),
                )
            y_sb = xpool.tile([1, BC], dt.float32, tag="ysb")
            nc.scalar.copy(y_sb[:], yac[:])
            nc.sync.dma_start(y[:], y_sb[:])

    nc.finalize()
    return nc


def _get_nc():
    if "nc" not in _STATE:
        _STATE["nc"] = _build_nc()
    return _STATE["nc"]


def _pack_w0(W0):
    # fold symmetric (f, c) weight pairs onto f <= c; pad to K0 with zeros
    w = np.asarray(W0, np.float32).reshape(O, F, F)
    wp = np.zeros((O, K0), np.float32)
    k = 0
    for f in range(F):
        wp[:, k] = w[:, f, f]
        k += 1
        n = F - f - 1
        if n:
            wp[:, k : k + n] = w[:, f, f + 1 :] + w[:, f + 1 :, f]
            k += n
    return wp


def _prep_in_maps(x, W0, W1, W2, Wa):
    x = np.asarray(x, dtype=np.float32)

    def w_layout(wt):
        # (K, O) -> (128, KT*O): row p holds [Wt[128t+p, :] for t in 0..KT)
        K = wt.shape[0]
        return np.ascontiguousarray(
            wt.reshape(K // 128, 128, O).transpose(1, 0, 2).reshape(128, -1)
        )

    w0t = w_layout(_pack_w0(W0).T).astype(_BF16)
    w1t = w_layout(np.ascontiguousarray(np.asarray(W1, np.float32).T)).astype(_BF16)
    w2t = w_layout(np.ascontiguousarray(np.asarray(W2, np.float32).T)).astype(_BF16)
    wa = np.ascontiguousarray(np.asarray(Wa, np.float32).reshape(3, O).T)
    def pack_gather(x0b, idx):
        g = x0b[idx]                                        # (K0, J)
        g = g.reshape(KT0, 128, NJ, JB).transpose(2, 1, 0, 3)
        return np.ascontiguousarray(g.reshape(NJ, 128, KT0 * JB))

    in_maps = []
    for c in range(NCORES):
        xc = x[c * BC : (c + 1) * BC]                       # (BC, F, E)
        x0 = np.ascontiguousarray(xc.transpose(1, 0, 2).reshape(F, J))
        x0b = x0.astype(_BF16)
        # seed blocks: x0seed[4*jj+hx] = x0[16hx:16hx+16, jj-block] flattened,
        # replicated across 32 partitions
        x0r = x0b.reshape(F, NJ, JB)
        seeds = np.empty((NJ * 4, 32, 16 * JB), _BF16)
        for jj in range(NJ):
            for hx in range(4):
                blk = x0r[16 * hx : 16 * hx + 16, jj].reshape(1, 16 * JB)
                seeds[4 * jj + hx] = np.broadcast_to(blk, (32, 16 * JB))
        in_maps.append(
            {
                "x0seed": seeds,
                "x0packf": pack_gather(x0b, _F_IDX),
                "x0packc": pack_gather(x0b, _C_IDX),
                "w0t": w0t,
                "w1t": w1t,
                "w2t": w2t,
                "wa": wa,
            }
        )
    return in_maps


def _run(inputs, trace=False, **kwargs):
    from concourse.bass_utils import run_bass_kernel_spmd

    nc = _get_nc()
    in_maps = _prep_in_maps(**inputs)
    res = run_bass_kernel_spmd(
        nc, in_maps, core_ids=list(range(NCORES)), trace=trace, **kwargs
    )
    y = np.concatenate(
        [np.asarray(r["y"], np.float32).reshape(BC) for r in res.results]
    )
    return y, res


def kernel(**inputs) -> np.ndarray:
    y, _ = _run(inputs, trace=False)
    return y


# revision 31
# speedup vs baseline: 1.3465x; 1.2672x over previous
"""CIN (Compressed Interaction Network) kernel for Trainium2, 8 NeuronCores.

Reference computation (per sample b, NFIELD=64, NEMB=64, NFILTER=128, 3 layers):
    xk_{l+1}[o, e] = relu( sum_{f,c} W_l[o, f*C+c] * x0[f, e] * xk_l[c, e] )
    pooled_l = sum_e xk_{l+1};  y = concat(pooled) @ Wa.T

Strategy:
  - Data-parallel over batch: 32 samples/core, free axis J = 32*64 = 2048 (b-major,
    e-minor). Columns are independent through all layers; only the final pooled
    sum groups by b.
  - Per layer the GEMM is out = W @ H with H[(f,c), j] = x0[f,j] * xk[c,j]
    (Khatri-Rao column structure). H is materialized K-tile by K-tile in bf16 by
    DVE tensor_tensor with plain 2D unit-stride APs (DVE 2x_1P perf mode).
  - Layer 0 is symmetric (xk = x0): W0 is host-folded onto upper-triangle
    (f<=c) pairs, K = 2080 -> 17 K-tiles (vs 32), and both TT operands are
    host-gathered arrays (x0pack_f/x0pack_c) loaded straight from DRAM.
  - Layers 1-2: the x0_f modulator rows are partition-replicated in "hex"
    tiles of 16 fields: one seed DMA (32 partitions, from a host-side
    32x-replicated x0rep32) + 2 partition-doubling SBUF->SBUF DMAs.
    DMA issue is wave-interleaved across hex tiles to avoid FIFO
    head-of-line blocking on the Sync queue.
  - PE runs bf16 matmuls (N=512) accumulating in PSUM; ScalarE applies ReLU
    4x into a repeated next-layer input xk4; VectorE reduces pooled in fp32.
  - Weights host-pre-transposed to (K, O) bf16; W1/W2 loaded via the GPSIMD
    DMA queue so they don't block the Sync queue at startup.
"""

import sys

if "/opt/trn_rl_repo" not in sys.path:
    sys.path.insert(0, "/opt/trn_rl_repo")

import numpy as np
import ml_dtypes

B, F, E, O = 256, 64, 64, 128
NCORES = 8
BC = B // NCORES          # samples per core
J = BC * E                # free columns per core
JB = 512                  # free-block size (one PSUM bank)
NJ = J // JB              # 4 free blocks
KT0 = 17                  # layer-0 K-tiles (packed symmetric, 2176 = 17*128)
K0 = KT0 * 128
KT = [KT0, 64, 64]

_BF16 = ml_dtypes.bfloat16
_STATE = {}

# layer-0 packed pair enumeration (f <= c), padded to K0 with (0, 0)
_PAIRS = [(f, c) for f in range(F) for c in range(f, F)]
_F_IDX = np.array([p[0] for p in _PAIRS] + [0] * (K0 - len(_PAIRS)), np.int64)
_C_IDX = np.array([p[1] for p in _PAIRS] + [0] * (K0 - len(_PAIRS)), np.int64)


def _build_nc():
    import concourse.bass as bass
    import concourse.tile as tile
    import concourse.mybir as mybir
    from concourse import bacc

    dt = mybir.dt
    nc = bacc.Bacc("TRN2", target_bir_lowering=False, debug=False)

    # host-relayout arrays: contiguous per DMA destination partition
    x0seed = nc.dram_tensor(
        "x0seed", [NJ * 4, 32, 16 * JB], dt.bfloat16, kind="ExternalInput"
    )
    x0packf = nc.dram_tensor(
        "x0packf", [NJ, 128, KT0 * JB], dt.bfloat16, kind="ExternalInput"
    )
    x0packc = nc.dram_tensor(
        "x0packc", [NJ, 128, KT0 * JB], dt.bfloat16, kind="ExternalInput"
    )
    # weights pre-laid-out as [partition, ktile*O] (contiguous per partition)
    w0t = nc.dram_tensor("w0t", [128, KT0 * O], dt.bfloat16, kind="ExternalInput")
    w1t = nc.dram_tensor("w1t", [128, 64 * O], dt.bfloat16, kind="ExternalInput")
    w2t = nc.dram_tensor("w2t", [128, 64 * O], dt.bfloat16, kind="ExternalInput")
    wa = nc.dram_tensor("wa", [O, 3], dt.float32, kind="ExternalInput")
    y = nc.dram_tensor("y", [1, BC], dt.float32, kind="ExternalOutput")

    HEXW = 16 * JB            # free width of a 16-field modulator tile
    PKW = KT0 * JB            # free width of a packed layer-0 operand tile

    with tile.TileContext(nc) as tc:
        with (
            tc.tile_pool(name="wpool", bufs=1) as wpool,
            tc.tile_pool(name="xpool", bufs=1) as xpool,
            tc.tile_pool(name="modpool", bufs=5) as modpool,
            tc.tile_pool(name="packpool", bufs=1) as packpool,
            tc.tile_pool(name="hpool", bufs=6) as hpool,
            tc.tile_pool(name="xkpool", bufs=2) as xkpool,
            tc.tile_pool(name="psum", bufs=2, space="PSUM") as psum_pool,
            tc.tile_pool(name="psumy", bufs=1, space="PSUM") as psumy_pool,
            tc.tile_pool(name="psumw", bufs=1, space="PSUM") as psumw_pool,
        ):
            # --- static loads -------------------------------------------------
            wa_sb = xpool.tile([O, 3], dt.float32, tag="wa")
            nc.sync.dma_start(wa_sb[:], wa[:])
            w_sb = []
            for li, (wd, kt) in enumerate(zip((w0t, w1t, w2t), KT)):
                w = wpool.tile([128, kt, O], dt.bfloat16, tag=f"w{li}", name=f"w{li}")
                nc.sync.dma_start(w[:].rearrange("p t o -> p (t o)"), wd[:])
                w_sb.append(w)
            pooled = [
                xpool.tile([O, BC], dt.float32, tag=f"pooled{l}", name=f"pooled{l}")
                for l in range(3)
            ]

            # --- main loop over free blocks ----------------------------------
            # scratch target for tiny keep-warm matmuls: the PE's HAM clock
            # gate re-throttles to 1.2 GHz on micro-idles between H-tile
            # bursts; a small filler matmul in each gap keeps it at 2.4 GHz
            warm_ps = psumw_pool.tile([64, 64], dt.float32, tag="warm")

            def warm_mm():
                nc.tensor.matmul(
                    warm_ps[:], w_sb[0][:, 0, 0:64], w_sb[0][:, 0, 0:64],
                    start=True, stop=True,
                )

            for jj in range(NJ):
                jsl = slice(JB * jj, JB * (jj + 1))
                # layer-0 packed operand tiles (contiguous per-partition loads)
                p0f = packpool.tile([128, PKW], dt.bfloat16, tag="p0f", name=f"p0f{jj}")
                p0c = packpool.tile([128, PKW], dt.bfloat16, tag="p0c", name=f"p0c{jj}")
                nc.sync.dma_start(p0f[:], x0packf[jj])
                nc.sync.dma_start(p0c[:], x0packc[jj])
                # modulator hex tiles for layers 1-2:
                #   mh[p, 512*i + e] = x0[16*hx + i, jsl][e] for every p
                # chain-major issue: each hex's seed + doublings together, so a
                # chain blocked on its slot doesn't head-of-line-block the next
                # chain's transfers on the Sync queue.
                mhs = []
                for hx in range(4):
                    mh = modpool.tile(
                        [128, HEXW], dt.bfloat16, tag="mod", name=f"mh{jj}_{hx}"
                    )
                    nc.sync.dma_start(mh[0:32, :], x0seed[4 * jj + hx])
                    nc.sync.dma_start(mh[32:64, :], mh[0:32, :])
                    nc.sync.dma_start(mh[64:128, :], mh[0:64, :])
                    mhs.append(mh)

                xk4 = None
                for l in range(3):
                    kt = KT[l]
                    acc = psum_pool.tile(
                        [128, JB], dt.float32, tag="acc", name=f"acc{jj}_{l}"
                    )
                    if l == 0:
                        # 8 pair ops + 1 single op over 17 packed K-tiles
                        for s in range(9):
                            nk = 2 if s < 8 else 1
                            h = hpool.tile(
                                [128, 4 * JB], dt.bfloat16, tag="h", name=f"h0_{jj}_{s}"
                            )
                            w_ = JB * nk
                            nc.vector.tensor_tensor(
                                h[:, 0:w_],
                                p0c[:, 2 * JB * s : 2 * JB * s + w_],
                                p0f[:, 2 * JB * s : 2 * JB * s + w_],
                                op=mybir.AluOpType.mult,
                            )
                            for i in range(nk):
                                t = 2 * s + i
                                nc.tensor.matmul(
                                    acc[:], w_sb[0][:, t, :],
                                    h[:, JB * i : JB * (i + 1)],
                                    start=(t == 0), stop=(t == kt - 1),
                                )
                            warm_mm()
                    else:
                        for hx in range(4):
                            for s in range(4):
                                h = hpool.tile(
                                    [128, 4 * JB], dt.bfloat16, tag="h",
                                    name=f"h{jj}_{l}_{hx}_{s}",
                                )
                                if hx == 0 and s == 0:
                                    # first quad: per-K-tile ops reading only
                                    # the first xk4 slice, so the layer starts
                                    # after one ReLU slice instead of four
                                    for i in range(4):
                                        nc.vector.tensor_tensor(
                                            h[:, JB * i : JB * (i + 1)],
                                            xk4[:, 0:JB],
                                            mhs[0][:, JB * i : JB * (i + 1)],
                                            op=mybir.AluOpType.mult,
                                        )
                                else:
                                    nc.vector.tensor_tensor(
                                        h[:], xk4[:],
                                        mhs[hx][:, 4 * JB * s : 4 * JB * (s + 1)],
                                        op=mybir.AluOpType.mult,
                                    )
                                for i in range(4):
                                    t = 16 * hx + 4 * s + i
                                    nc.tensor.matmul(
                                        acc[:], w_sb[l][:, t, :],
                                        h[:, JB * i : JB * (i + 1)],
                                        start=(t == 0), stop=(t == kt - 1),
                                    )
                                warm_mm()
                    # epilogue: relu 4x into xk4 (repeated next-layer input);
                    # the last layer only feeds pooled, one slice suffices
                    xk4_new = xkpool.tile(
                        [128, 4 * JB], dt.bfloat16, tag="xk4", name=f"xk4_{jj}_{l}"
                    )
                    for i in range(4 if l < 2 else 1):
                        nc.scalar.activation(
                            xk4_new[:, JB * i : JB * (i + 1)], acc[:],
                            mybir.ActivationFunctionType.Relu,
                        )
                    nc.vector.tensor_reduce(
                        pooled[l][:, 8 * jj : 8 * jj + 8],
                        xk4_new[:, 0:JB].rearrange("p (b e) -> p b e", e=E),
                        axis=mybir.AxisListType.X,
                        op=mybir.AluOpType.add,
                    )
                    xk4 = xk4_new

            # --- head: y[b] = sum_l wa[:, l] . pooled[l][:, b] ----------------
            yac = psumy_pool.tile([1, BC], dt.float32, tag="yac")
            for l in range(3):
                nc.tensor.matmul(
                    yac[:], wa_sb[:, l : l + 1], pooled[l][:],
                    start=(l == 0), stop=(l == 2),
                )
            y_sb = xpool.tile([1, BC], dt.float32, tag="ysb")
            nc.scalar.copy(y_sb[:], yac[:])
            nc.sync.dma_start(y[:], y_sb[:])

    nc.finalize()
    return nc


def _get_nc():
    if "nc" not in _STATE:
        _STATE["nc"] = _build_nc()
    return _STATE["nc"]


def _pack_w0(W0):
    # fold symmetric (f, c) weight pairs onto f <= c; pad to K0 with zeros
    w = np.asarray(W0, np.float32).reshape(O, F, F)
    wp = np.zeros((O, K0), np.float32)
    k = 0
    for f in range(F):
        wp[:, k] = w[:, f, f]
        k += 1
        n = F - f - 1
        if n:
            wp[:, k : k + n] = w[:, f, f + 1 :] + w[:, f + 1 :, f]
            k += n
    return wp


def _prep_in_maps(x, W0, W1, W2, Wa):
    x = np.asarray(x, dtype=np.float32)

    def w_layout(wt):
        # (K, O) -> (128, KT*O): row p holds [Wt[128t+p, :] for t in 0..KT)
        K = wt.shape[0]
        return np.ascontiguousarray(
            wt.reshape(K // 128, 128, O).transpose(1, 0, 2).reshape(128, -1)
        )

    w0t = w_layout(_pack_w0(W0).T).astype(_BF16)
    w1t = w_layout(np.ascontiguousarray(np.asarray(W1, np.float32).T)).astype(_BF16)
    w2t = w_layout(np.ascontiguousarray(np.asarray(W2, np.float32).T)).astype(_BF16)
    wa = np.ascontiguousarray(np.asarray(Wa, np.float32).reshape(3, O).T)
    def pack_gather(x0b, idx):
        g = x0b[idx]                                        # (K0, J)
        g = g.reshape(KT0, 128, NJ, JB).transpose(2, 1, 0, 3)
        return np.ascontiguousarray(g.reshape(NJ, 128, KT0 * JB))

    in_maps = []
    for c in range(NCORES):
        xc = x[c * BC : (c + 1) * BC]                       # (BC, F, E)
        x0 = np.ascontiguousarray(xc.transpose(1, 0, 2).reshape(F, J))
        x0b = x0.astype(_BF16)
        # seed blocks: x0seed[4*jj+hx] = x0[16hx:16hx+16, jj-block] flattened,
        # replicated across 32 partitions
        x0r = x0b.reshape(F, NJ, JB)
        seeds = np.empty((NJ * 4, 32, 16 * JB), _BF16)
        for jj in range(NJ):
            for hx in range(4):
                blk = x0r[16 * hx : 16 * hx + 16, jj].reshape(1, 16 * JB)
                seeds[4 * jj + hx] = np.broadcast_to(blk, (32, 16 * JB))
        in_maps.append(
            {
                "x0seed": seeds,
                "x0packf": pack_gather(x0b, _F_IDX),
                "x0packc": pack_gather(x0b, _C_IDX),
                "w0t": w0t,
                "w1t": w1t,
                "w2t": w2t,
                "wa": wa,
            }
        )
    return in_maps


def _run(inputs, trace=False, **kwargs):
    from concourse.bass_utils import run_bass_kernel_spmd

    nc = _get_nc()
    in_maps = _prep_in_maps(**inputs)
    res = run_bass_kernel_spmd(
        nc, in_maps, core_ids=list(range(NCORES)), trace=trace, **kwargs
    )
    y = np.concatenate(
        [np.asarray(r["y"], np.float32).reshape(BC) for r in res.results]
    )
    return y, res


def kernel(**inputs) -> np.ndarray:
    y, _ = _run(inputs, trace=False)
    return y
